# revision 1
# baseline (speedup 1.0000x reference)
"""Conv1d (K=5, pad=2) with folded LoRA on 8 Trainium2 NeuronCores.

Strategy
--------
Data-parallel: batch 8 -> 1 batch item per core. The LoRA path is folded
into the conv weights on the host (exact up to fp32 rounding):
    W_eff = conv_w + (alpha/rank) * einsum('or,rik->oik', lora_B, lora_A)
so the device kernel is a single conv1d + bias.

Per core: y[co, t] = bias[co] + sum_{k,ci} W_eff[co, ci, k] * x[ci, t+k-2]
computed as 5 shifted matmuls accumulating in PSUM, over 2 ci-blocks and
2 co-blocks of 128, in fp32r (TF32-class PE mode, 1 cycle/row; ~1.5e-4
scale-relative absmax at K=128 contraction, measured on HW).

Toolchain constraint baked into the structure: every instruction may carry
at most ONE sync wait (walrus setupSyncWait limit), and Tile's wait elision
is per-proc (engine vs sequencer are distinct procs, no transitivity).
Hence:
  - PE "observer" matmuls (1-column, scratch PSUM) absorb each x/weight DMA
    lane wait so real matmuls only wait on the DVE sem (PSUM-bank WAR).
  - Evictions (PSUM->SBUF + bias add) run exclusively on DVE and wait only
    on PE; out-DMA-slot WAR is absorbed by tiny DVE memsets; the bias lane
    by a tiny DVE copy.
  - x-loads ride the SP HWDGE ring, stores the ACT HWDGE ring; same-ring
    WAW lane waits are absorbed by sequencer nops on the matching ring.
  - A tail chain of 1-dep sync nops covers all procs so the exit drain
    carries at most one wait.
"""
import sys
sys.path.insert(0, "/opt/trn_rl_repo")
import numpy as np

from concourse import bass, mybir, tile
from concourse import bass_utils
from concourse.tile import add_dep_helper

# Problem constants (hardcoded per contract)
B = 8
CI = 256
CO = 256
K = 5
PAD = 2
T = 16384
RANK = 8
ALPHA = 16.0
SCALING = ALPHA / RANK
N_CORES = 8

# Tiling
CHUNK = 1024          # output columns per chunk
NCHUNK = T // CHUNK   # 16
SUB = 512             # matmul free dim
NSUB = CHUNK // SUB   # 2
XCOLS = CHUNK + 2 * PAD  # chunk + halo


def _build_nc(reps=1):
    f32 = mybir.dt.float32
    f32r = mybir.dt.float32r

    nc = bass.Bass(trn_type="TRN2", debug=False)
    x = nc.dram_tensor("x", [CI, T], f32, kind="ExternalInput").ap()
    wts = nc.dram_tensor("wts", [128, K * 2 * 2 * 128], f32, kind="ExternalInput").ap()
    bias = nc.dram_tensor("bias", [128, 2], f32, kind="ExternalInput").ap()
    zeros = nc.dram_tensor("zeros", [128, 2, PAD], f32, kind="ExternalInput").ap()
    # one output tensor per 2048-wide super-chunk, stored via SWDGE so each
    # store owns a DMASW lane exactly once (no lane-predecessor wait); host
    # concatenates
    ys = [nc.dram_tensor(f"y{s}", [CI, 2 * CHUNK], f32, kind="ExternalOutput").ap()
          for s in range(NCHUNK // 2)]

    xab = x.rearrange("(b p) t -> p b t", p=128)
    ysab = [yc.rearrange("(b p) t -> p b t", p=128) for yc in ys]

    NOB = 2   # out staging buffers (super-chunks)
    NPB = 6   # psum accumulation banks

    with tile.TileContext(nc) as tc:
        with tc.tile_pool(name="wp", bufs=1) as wp, \
             tc.tile_pool(name="pp", bufs=1, space="PSUM") as pp:

            # write-once observer scratch: two columns per observer matmul
            # (fp32r APs need 8-byte alignment)
            obs_ps = pp.tile([128, 64], f32, name="obs_ps", tag="obs")
            pbufs = [pp.tile([128, SUB], f32, name=f"pt{j}", tag=f"pt{j}")
                     for j in range(NPB)]
            # x is fully resident: one dedicated buffer per chunk, no reuse
            xbufs = [wp.tile([128, 2, XCOLS], f32r, name=f"xt{j}", tag=f"xt{j}")
                     for j in range(NCHUNK)]
            obufs = [wp.tile([128, 2, 2 * CHUNK], f32, name=f"ot{j}", tag=f"ot{j}")
                     for j in range(NOB)]
            # write-once DVE gate scratch: one column per gate memset
            gs = wp.tile([128, 4 * NCHUNK * reps + 8], f32, name="gs")

            wr = wp.tile([128, K * 2 * 2 * 128], f32r, name="wr")
            d_w = nc.sync.dma_start(out=wr[:], in_=wts[:].bitcast(f32r))
            bs = wp.tile([128, 2], f32, name="bs")
            d_b = nc.sync.dma_start(out=bs[:], in_=bias[:])

            n_obs = [0]

            def pe_observe(src_ap, dma_inst):
                """1-column matmul whose only wait is `dma_inst`'s lane.

                Reads only within the region `dma_inst` wrote; writes its own
                never-reused obs_ps column (no WAW chain)."""
                n = src_ap.shape[-1]
                m = min(128, n)
                oc = 2 * n_obs[0]
                n_obs[0] += 1
                mm = nc.tensor.matmul(obs_ps[0:m, oc:oc + 2], src_ap[:, 0:m],
                                      src_ap[:, 0:2], start=True, stop=True)
                add_dep_helper(mm.ins, dma_inst.ins, sync=False, reason="obs-order")
                return mm

            n_gate = [0]

            def dve_gate(dep_inst):
                """Write-once DVE memset whose only wait is dep's proc tick."""
                gc = n_gate[0]
                n_gate[0] += 1
                ms = nc.vector.memset(gs[:, gc:gc + 1], 0.0)
                add_dep_helper(ms.ins, dep_inst.ins, sync=True, reason="dve-gate")
                return ms

            obs_w = pe_observe(wr, d_w)
            # DVE observes the bias lane via a write-once copy
            bscratch = wp.tile([128, 2], f32, name="bscratch")
            obs_b = nc.vector.tensor_copy(bscratch[:], bs[:])

            in_dmas = []      # list of lists per chunk
            out_dmas = []     # per super-chunk (final rep only)
            sc_evicts = {}    # global super-chunk -> last evict
            sc_ods = {}       # global super-chunk -> out dma
            last_mm = None
            last_evict = None
            pi = 0            # psum bank rotation
            NSC = NCHUNK // 2

            for r in range(reps):
                for c in range(NCHUNK):
                    lo = c * CHUNK - PAD
                    xt = xbufs[c]

                    observers = []
                    if r == 0:
                        chunk_dmas = []
                        if c == 0:
                            chunk_dmas.append(nc.sync.dma_start(
                                out=xt[:, :, PAD:XCOLS],
                                in_=xab[:, :, 0:CHUNK + PAD].bitcast(f32r)))
                            chunk_dmas.append(nc.sync.dma_start(
                                out=xt[:, :, 0:PAD], in_=zeros[:].bitcast(f32r)))
                        elif c == NCHUNK - 1:
                            chunk_dmas.append(nc.sync.dma_start(
                                out=xt[:, :, 0:CHUNK + PAD],
                                in_=xab[:, :, lo:T].bitcast(f32r)))
                            chunk_dmas.append(nc.sync.dma_start(
                                out=xt[:, :, CHUNK + PAD:XCOLS],
                                in_=zeros[:].bitcast(f32r)))
                        else:
                            chunk_dmas.append(nc.sync.dma_start(
                                out=xt[:], in_=xab[:, :, lo:lo + XCOLS].bitcast(f32r)))
                        in_dmas.append(chunk_dmas)

                        # PE observes this chunk's x lanes via 1-col matmuls;
                        # each observer reads only within its DMA's region.
                        for i, d in enumerate(chunk_dmas):
                            if i == 0:
                                src_ap = (xt[:, 0, PAD:PAD + 128] if c == 0
                                          else xt[:, 0, 0:128])
                            else:
                                src_ap = (xt[:, 0, 0:PAD] if c == 0
                                          else xt[:, 0, CHUNK + PAD:XCOLS])
                            observers.append(pe_observe(src_ap, d))

                    sc, half = divmod(c, 2)
                    gsc = r * NSC + sc
                    ot = obufs[gsc % NOB]
                    evict_gates = [obs_b]
                    if half == 0 and gsc >= NOB:
                        # pre-lift the recycled out buffer's history onto
                        # DVE's observed clock: one 1-wait gate per proc
                        old = gsc - NOB
                        evict_gates.append(dve_gate(sc_evicts[old]))
                        if old in sc_ods:
                            evict_gates.append(dve_gate(sc_ods[old]))

                    first_evict_of_chunk = True
                    for co in range(2):
                        for ts in range(NSUB):
                            pt = pbufs[pi % NPB]
                            pi += 1
                            first = True
                            for b in range(2):
                                for k in range(K):
                                    widx = ((k * 2 + b) * 2 + co) * 128
                                    mm = nc.tensor.matmul(
                                        pt[:],
                                        wr[:, widx:widx + 128],
                                        xt[:, b, ts * SUB + k: ts * SUB + k + SUB],
                                        start=first,
                                        stop=(b == 1 and k == K - 1),
                                    )
                                    if first:
                                        for ob in observers:
                                            add_dep_helper(
                                                mm.ins, ob.ins, sync=False,
                                                reason="order-after-observe")
                                    first = False
                                    last_mm = mm
                            off = half * CHUNK + ts * SUB
                            ev = nc.vector.tensor_scalar_add(
                                out=ot[:, co, off:off + SUB],
                                in0=pt[:],
                                scalar1=bs[:, co:co + 1],
                            )
                            if first_evict_of_chunk:
                                for g in evict_gates:
                                    add_dep_helper(ev.ins, g.ins, sync=False,
                                                   reason="order-after-gate")
                                first_evict_of_chunk = False
                            last_evict = ev

                    if half == 1:
                        sc_evicts[gsc] = last_evict
                        if r == reps - 1:
                            # SWDGE store: own output tensor + own DMASW lane
                            od = nc.gpsimd.dma_start(out=ysab[sc][:], in_=ot[:])
                            sc_ods[gsc] = od
                            out_dmas.append(od)

            # Tail flush: cover every proc with 1-dep sync nops so the final
            # drain carries at most one wait.
            tail_deps = [d for ds in in_dmas[-8:] for d in ds] + out_dmas + \
                [last_mm, last_evict]
            for dep in tail_deps:
                nop = nc.sync.nop()
                add_dep_helper(nop.ins, dep.ins, sync=True, reason="tailflush")

    return nc


def check_waits(nc):
    """Return instructions carrying more than one sync wait (walrus limit)."""
    bad = []
    for f in nc.m.functions:
        for bb in f.blocks:
            for inst in bb.instructions:
                si = inst.sync_info
                nw = len(si.on_wait) if si and si.on_wait else 0
                if nw > 1:
                    bad.append((inst.name, type(inst).__name__, nw,
                                [w.ant_name for w in si.on_wait]))
    return bad


def _pack_weights(conv_w, conv_b, lora_A, lora_B):
    w_eff = conv_w.astype(np.float32) + (
        SCALING * np.einsum(
            "or,rik->oik", lora_B.astype(np.float64),
            lora_A.astype(np.float64).reshape(RANK, CI, K))
    ).astype(np.float32)
    # wts[ci_in_block, ((k*2 + b)*2 + co)*128 + m] = w_eff[co*128+m, b*128+ci, k]
    a = w_eff.reshape(2, 128, 2, 128, K)        # [co_blk, m, ci_blk, ci, k]
    a = a.transpose(3, 4, 2, 0, 1)              # [ci, k, b, co_blk, m]
    wts = np.ascontiguousarray(a.reshape(128, K * 2 * 2 * 128), dtype=np.float32)
    bias = np.ascontiguousarray(
        conv_b.astype(np.float32).reshape(2, 128).T)  # [128, 2]
    return wts, bias


_CACHED_NC = None


def kernel(x, conv_w, conv_b, lora_A, lora_B, _trace=False):
    global _CACHED_NC
    x = np.asarray(x, dtype=np.float32)
    wts, bias = _pack_weights(np.asarray(conv_w), np.asarray(conv_b),
                              np.asarray(lora_A), np.asarray(lora_B))
    zeros = np.zeros((128, 2, PAD), dtype=np.float32)

    if _CACHED_NC is None:
        _CACHED_NC = _build_nc()
        bad = check_waits(_CACHED_NC)
        assert not bad, f"sync-wait violations: {bad[:5]}"
    nc = _CACHED_NC

    in_maps = [
        {"x": x[i], "wts": wts, "bias": bias, "zeros": zeros}
        for i in range(N_CORES)
    ]
    res = bass_utils.run_bass_kernel_spmd(
        nc, in_maps, core_ids=list(range(N_CORES)), trace=_trace)
    out = np.stack(
        [np.concatenate([res.results[i][f"y{s}"] for s in range(NCHUNK // 2)],
                        axis=1)
         for i in range(N_CORES)], axis=0)
    if _trace:
        kernel._last_exec_time_ns = res.exec_time_ns
        kernel._last_results = res
    return out


if __name__ == "__main__":
    nc = _build_nc()
    bad = check_waits(nc)
    print("violations:", bad[:10])
    n_inst = sum(len(bb.instructions) for f in nc.m.functions for bb in f.blocks)
    print("instructions:", n_inst)



# revision 2
# speedup vs baseline: 1.7943x; 1.7943x over previous
"""Conv1d (K=5, pad=2) with folded LoRA on 8 Trainium2 NeuronCores.

Strategy
--------
Data-parallel: batch 8 -> 1 batch item per core. LoRA is folded into the
conv weights on the host:
    w_eff = conv_w + (alpha/rank) * einsum('or,rik->oik', lora_B, lora_A)

The device kernel runs entirely in fp8-e4m3 DoubleRow matmuls (0.5
cycles/output-column with a 256-wide contraction -- 4x the per-column fp32r
rate). Precision is recovered with a hi/lo split computed on the host:

    x_hi = e4m3(x)                  x_lo  = e4m3(x - x_hi)
    W16  = e4m3(16*w_eff)           WCOR  = e4m3(16*(w_eff - W16/16))

    psum = W16@x_hi + W16@x_lo + WCOR@x_hi        (WCOR only for the
    y    = psum/16 + bias                          N_CORR highest-energy taps)

Per psum tile [128co, 512t]: 5 hi + 5 lo + N_CORR correction DoubleRow
matmuls, each pairing the two ci-blocks in the DoubleRow slots. Eviction is
one DVE tensor_scalar affine (psum * 1/16 + bias) straight to fp16 staging;
outputs travel as fp16 and are upcast on the host. Measured end-to-end
rel-err vs the fp64 reference: 1.9e-2 @ N_CORR=2 (gate 2e-2), 1.7e-2 @ 3.

Toolchain constraint baked into the structure: every instruction may carry
at most ONE sync wait (walrus setupSyncWait limit), and Tile's wait elision
is per-proc. Hence (same architecture as the fp32r predecessor):
  - PE "observer" matmuls (tiny, scratch PSUM) absorb each x/weight DMA
    lane wait so real matmuls only wait on the DVE sem (PSUM-bank WAR).
  - Evictions run exclusively on DVE and wait only on PE; out-DMA-slot WAR
    is absorbed by tiny DVE memsets; the bias lane by a tiny DVE copy.
  - x-loads ride the SP HWDGE ring, stores the SWDGE path (own DMASW lane
    per store via one output tensor per super-chunk).
  - A tail chain of 1-dep sync nops covers all procs so the exit drain
    carries at most one wait.
"""
import sys
sys.path.insert(0, "/opt/trn_rl_repo")
import numpy as np
import ml_dtypes

from concourse import bass, mybir, tile
from concourse import bass_utils
from concourse.tile import add_dep_helper

E4M3 = ml_dtypes.float8_e4m3fn

# Problem constants (hardcoded per contract)
B = 8
CI = 256
CO = 256
K = 5
PAD = 2
T = 16384
RANK = 8
ALPHA = 16.0
SCALING = ALPHA / RANK
N_CORES = 8

N_CORR = 2            # correction taps (w-error fix); 2 -> rel_err ~1.9e-2

# Tiling
CHUNK = 1024          # output columns per chunk
NCHUNK = T // CHUNK   # 16
SUB = 512             # matmul free dim (one PSUM bank)
NSUB = CHUNK // SUB   # 2
XCOLS = CHUNK + 2 * PAD  # chunk + halo


def _build_nc(corr_taps):
    f32 = mybir.dt.float32
    f16 = mybir.dt.float16
    f8 = mybir.dt.float8e4
    DR = mybir.MatmulPerfMode.DoubleRow
    n_corr = len(corr_taps)

    nc = bass.Bass(trn_type="TRN2", debug=False)
    # x slots: 0,1 = x_hi(ci blk 0/1); 2,3 = x_lo(ci blk 0/1)
    x = nc.dram_tensor("x", [128, 4, T], f8, kind="ExternalInput").ap()
    wts = nc.dram_tensor("wts", [128, K * 2 * 2 * 128], f8,
                         kind="ExternalInput").ap()
    bias = nc.dram_tensor("bias", [128, 2], f32, kind="ExternalInput").ap()
    zeros = nc.dram_tensor("zeros", [128, 4, PAD], f8, kind="ExternalInput").ap()
    if n_corr:
        wcor = nc.dram_tensor("wcor", [128, n_corr * 2 * 2 * 128], f8,
                              kind="ExternalInput").ap()
    # one output tensor per 2048-wide super-chunk, stored via SWDGE so each
    # store owns a DMASW lane exactly once; host concatenates
    ys = [nc.dram_tensor(f"y{s}", [128, 2, 2 * CHUNK], f16,
                         kind="ExternalOutput").ap()
          for s in range(NCHUNK // 2)]

    NOB = 2   # out staging buffers (super-chunks)
    NPB = 6   # psum accumulation banks

    with tile.TileContext(nc) as tc:
        with tc.tile_pool(name="wp", bufs=1) as wp, \
             tc.tile_pool(name="pp", bufs=1, space="PSUM") as pp:

            # write-once observer scratch: four columns per observer matmul
            obs_ps = pp.tile([128, 96], f32, name="obs_ps", tag="obs")
            pbufs = [pp.tile([128, SUB], f32, name=f"pt{j}", tag=f"pt{j}")
                     for j in range(NPB)]
            # x is fully resident: one dedicated buffer per chunk, no reuse
            xbufs = [wp.tile([128, 4, XCOLS], f8, name=f"xt{j}", tag=f"xt{j}")
                     for j in range(NCHUNK)]
            obufs = [wp.tile([128, 2, 2 * CHUNK], f16, name=f"ot{j}",
                             tag=f"ot{j}")
                     for j in range(NOB)]
            # write-once DVE gate scratch: one column per gate memset
            gs = wp.tile([128, 4 * NCHUNK + 8], f32, name="gs")

            wt = wp.tile([128, K, 2, 2, 128], f8, name="wt")
            d_w = nc.sync.dma_start(
                out=wt[:],
                in_=wts[:].rearrange("p (k c i m) -> p k c i m", k=K, c=2, i=2))
            if n_corr:
                wc = wp.tile([128, n_corr, 2, 2, 128], f8, name="wc")
                d_wc = nc.sync.dma_start(
                    out=wc[:],
                    in_=wcor[:].rearrange("p (j c i m) -> p j c i m",
                                          j=n_corr, c=2, i=2))
            bs = wp.tile([128, 2], f32, name="bs")
            d_b = nc.sync.dma_start(out=bs[:], in_=bias[:])

            n_obs = [0]

            def pe_observe(src_ap, dma_inst):
                """Tiny matmul whose only wait is `dma_inst`'s lane.

                Reads only within the region `dma_inst` wrote; writes its own
                never-reused obs_ps columns (no WAW chain)."""
                n = src_ap.shape[-1]
                m = min(4, n)
                oc = 4 * n_obs[0]
                n_obs[0] += 1
                mm = nc.tensor.matmul(obs_ps[0:m, oc:oc + m], src_ap[:, 0:m],
                                      src_ap[:, 0:m], start=True, stop=True)
                add_dep_helper(mm.ins, dma_inst.ins, sync=False,
                               reason="obs-order")
                return mm

            n_gate = [0]

            def dve_gate(dep_inst):
                """Write-once DVE memset whose only wait is dep's proc tick."""
                gc = n_gate[0]
                n_gate[0] += 1
                ms = nc.vector.memset(gs[:, gc:gc + 1], 0.0)
                add_dep_helper(ms.ins, dep_inst.ins, sync=True,
                               reason="dve-gate")
                return ms

            obs_w = pe_observe(wt[:, 0, 0, 0], d_w)
            if n_corr:
                obs_wc = pe_observe(wc[:, 0, 0, 0], d_wc)
            # DVE observes the bias lane via a write-once copy
            bscratch = wp.tile([128, 2], f32, name="bscratch")
            obs_b = nc.vector.tensor_copy(bscratch[:], bs[:])

            in_dmas = []      # list of lists per chunk
            out_dmas = []     # per super-chunk
            sc_evicts = {}    # super-chunk -> last evict
            sc_ods = {}       # super-chunk -> out dma
            last_mm = None
            last_evict = None
            pi = 0            # psum bank rotation

            for c in range(NCHUNK):
                lo = c * CHUNK - PAD
                xt = xbufs[c]

                chunk_dmas = []
                if c == 0:
                    chunk_dmas.append(nc.sync.dma_start(
                        out=xt[:, :, PAD:XCOLS],
                        in_=x[:, :, 0:CHUNK + PAD]))
                    chunk_dmas.append(nc.sync.dma_start(
                        out=xt[:, :, 0:PAD], in_=zeros[:]))
                elif c == NCHUNK - 1:
                    chunk_dmas.append(nc.sync.dma_start(
                        out=xt[:, :, 0:CHUNK + PAD],
                        in_=x[:, :, lo:T]))
                    chunk_dmas.append(nc.sync.dma_start(
                        out=xt[:, :, CHUNK + PAD:XCOLS],
                        in_=zeros[:]))
                else:
                    chunk_dmas.append(nc.sync.dma_start(
                        out=xt[:], in_=x[:, :, lo:lo + XCOLS]))
                in_dmas.append(chunk_dmas)

                # PE observes this chunk's x lanes; each observer reads only
                # within its DMA's region.
                observers = []
                for i, d in enumerate(chunk_dmas):
                    if i == 0:
                        src_ap = (xt[:, 0, PAD:PAD + 4] if c == 0
                                  else xt[:, 0, 0:4])
                    else:
                        src_ap = (xt[:, 0, 0:PAD] if c == 0
                                  else xt[:, 0, CHUNK + PAD:XCOLS])
                    observers.append(pe_observe(src_ap, d))
                if c == 0:
                    observers += [obs_w] + ([obs_wc] if n_corr else [])

                sc, half = divmod(c, 2)
                ot = obufs[sc % NOB]
                evict_gates = [obs_b]
                if half == 0 and sc >= NOB:
                    # pre-lift the recycled out buffer's history onto DVE's
                    # observed clock: one 1-wait gate per proc
                    old = sc - NOB
                    evict_gates.append(dve_gate(sc_evicts[old]))
                    if old in sc_ods:
                        evict_gates.append(dve_gate(sc_ods[old]))

                first_evict_of_chunk = True
                for co in range(2):
                    for ts in range(NSUB):
                        pt = pbufs[pi % NPB]
                        pi += 1
                        mm_specs = (
                            [(wt[:, k, co], 0, k) for k in range(K)] +
                            [(wt[:, k, co], 2, k) for k in range(K)] +
                            [(wc[:, j, co], 0, k)
                             for j, k in enumerate(corr_taps)])
                        for n_i, (st, sl, k) in enumerate(mm_specs):
                            off = ts * SUB + k
                            mm = nc.tensor.matmul(
                                pt[:],
                                st,
                                xt[:, sl:sl + 2, off:off + SUB],
                                start=(n_i == 0),
                                stop=(n_i == len(mm_specs) - 1),
                                perf_mode=DR,
                            )
                            if n_i == 0:
                                for ob in observers:
                                    add_dep_helper(
                                        mm.ins, ob.ins, sync=False,
                                        reason="order-after-observe")
                            last_mm = mm
                        off = half * CHUNK + ts * SUB
                        ev = nc.vector.tensor_scalar(
                            out=ot[:, co, off:off + SUB],
                            in0=pt[:],
                            scalar1=1.0 / 16.0,
                            scalar2=bs[:, co:co + 1],
                            op0=mybir.AluOpType.mult,
                            op1=mybir.AluOpType.add,
                        )
                        if first_evict_of_chunk:
                            for g in evict_gates:
                                add_dep_helper(ev.ins, g.ins, sync=False,
                                               reason="order-after-gate")
                            first_evict_of_chunk = False
                        last_evict = ev

                if half == 1:
                    sc_evicts[sc] = last_evict
                    # SWDGE store: own output tensor + own DMASW lane
                    od = nc.gpsimd.dma_start(out=ys[sc][:], in_=ot[:])
                    sc_ods[sc] = od
                    out_dmas.append(od)

            # Tail flush: cover every proc with 1-dep sync nops so the final
            # drain carries at most one wait.
            tail_deps = [d for ds in in_dmas[-8:] for d in ds] + out_dmas + \
                [last_mm, last_evict]
            for dep in tail_deps:
                nop = nc.sync.nop()
                add_dep_helper(nop.ins, dep.ins, sync=True, reason="tailflush")

    return nc


def check_waits(nc):
    """Return instructions carrying more than one sync wait (walrus limit)."""
    bad = []
    for f in nc.m.functions:
        for bb in f.blocks:
            for inst in bb.instructions:
                si = inst.sync_info
                nw = len(si.on_wait) if si and si.on_wait else 0
                if nw > 1:
                    bad.append((inst.name, type(inst).__name__, nw,
                                [w.ant_name for w in si.on_wait]))
    return bad


def _q8(a):
    return np.asarray(a, dtype=np.float32).astype(E4M3)


def _pack_weights(conv_w, conv_b, lora_A, lora_B):
    w_eff = (conv_w.astype(np.float64) + SCALING * np.einsum(
        "or,rik->oik", lora_B.astype(np.float64),
        lora_A.astype(np.float64).reshape(RANK, CI, K))).astype(np.float32)
    W16 = _q8(16.0 * w_eff)
    w_lo = w_eff - W16.astype(np.float32) / 16.0
    energies = [(float(np.square(w_lo[:, :, k]).sum()), k) for k in range(K)]
    corr_taps = tuple(sorted(k for _, k in
                             sorted(energies, reverse=True)[:N_CORR]))
    WCOR = _q8(16.0 * w_lo)

    # wts[p, ((k*2 + c)*2 + i)*128 + m] = W16[c*128+m, i*128+p, k]
    def pack(w8, taps):
        a = w8.astype(np.float32).reshape(2, 128, 2, 128, K)  # [c, m, i, p, k]
        a = a[:, :, :, :, list(taps)]                          # [c, m, i, p, j]
        a = a.transpose(3, 4, 0, 2, 1)                         # [p, j, c, i, m]
        return np.ascontiguousarray(
            a.reshape(128, len(taps) * 2 * 2 * 128)).astype(E4M3)

    wts = pack(W16, range(K))
    wcor = pack(WCOR, corr_taps) if N_CORR else None
    bias = np.ascontiguousarray(
        conv_b.astype(np.float32).reshape(2, 128).T)  # [128, 2]
    return wts, wcor, bias, corr_taps


_CACHED = {}


def kernel(x, conv_w, conv_b, lora_A, lora_B, _trace=False):
    x = np.asarray(x, dtype=np.float32)
    wts, wcor, bias, corr_taps = _pack_weights(
        np.asarray(conv_w), np.asarray(conv_b),
        np.asarray(lora_A), np.asarray(lora_B))
    zeros = np.zeros((128, 4, PAD), dtype=E4M3)

    if corr_taps not in _CACHED:
        nc = _build_nc(corr_taps)
        bad = check_waits(nc)
        assert not bad, f"sync-wait violations: {bad[:5]}"
        _CACHED[corr_taps] = nc
    nc = _CACHED[corr_taps]
    # test.py compatibility handle
    kernel.__globals__["_CACHED_NC"] = nc

    x_hi = _q8(x)
    x_lo = _q8(x - x_hi.astype(np.float32))
    # xpack[core][p, s, t]; s = hl*2 + ci_blk
    xp = np.stack([x_hi.reshape(B, 2, 128, T), x_lo.reshape(B, 2, 128, T)],
                  axis=1)                       # [B, hl, blk, p, t]
    xp = np.ascontiguousarray(xp.transpose(0, 3, 1, 2, 4)  # [B, p, hl, blk, t]
                              .reshape(B, 128, 4, T))

    in_maps = []
    for i in range(N_CORES):
        m = {"x": xp[i], "wts": wts, "bias": bias, "zeros": zeros}
        if wcor is not None:
            m["wcor"] = wcor
        in_maps.append(m)
    res = bass_utils.run_bass_kernel_spmd(
        nc, in_maps, core_ids=list(range(N_CORES)), trace=_trace)
    outs = []
    for i in range(N_CORES):
        yc = np.concatenate([np.asarray(res.results[i][f"y{s}"])
                             for s in range(NCHUNK // 2)], axis=2)
        outs.append(yc.transpose(1, 0, 2).reshape(CO, T))
    out = np.stack(outs, axis=0).astype(np.float32)
    if _trace:
        kernel._last_exec_time_ns = res.exec_time_ns
        kernel._last_results = res
    return out


_CACHED_NC = None


if __name__ == "__main__":
    nc = _build_nc((0, 2))
    bad = check_waits(nc)
    print("violations:", bad[:10])
    n_inst = sum(len(bb.instructions) for f in nc.m.functions for bb in f.blocks)
    print("instructions:", n_inst)
    from concourse.timeline_sim import TimelineSim
    dur = TimelineSim(nc, trace=False).simulate()
    print(f"TimelineSim: {dur:.0f} ns")


# revision 37
# speedup vs baseline: 1.9200x; 1.0701x over previous
"""Conv1d (K=5, pad=2) with folded LoRA on 8 Trainium2 NeuronCores.

Strategy
--------
Data-parallel: batch 8 -> 1 batch item per core. LoRA is folded into the
conv weights on the host:
    w_eff = conv_w + (alpha/rank) * einsum('or,rik->oik', lora_B, lora_A)

The device kernel runs entirely in fp8-e4m3 DoubleRow matmuls (0.5
cycles/output-column with a 256-wide contraction -- 4x the per-column fp32r
rate). Precision is recovered with a hi/lo split computed on the host:

    x_hi = e4m3(x)                  x_lo  = e4m3(x - x_hi)
    W16  = e4m3(16*w_eff)           WCOR  = e4m3(16*(w_eff - W16/16))

    psum = W16@x_hi + W16@x_lo + WCOR@x_hi        (WCOR only for the
    y    = psum/16 + bias                          N_CORR highest-energy taps)

Per psum tile [128co, 512t]: 5 hi + 5 lo + N_CORR correction DoubleRow
matmuls, each pairing the two ci-blocks in the DoubleRow slots. Eviction is
one DVE tensor_scalar affine (psum * 1/16 + bias) straight to fp16 staging;
outputs travel as fp16 and are upcast on the host. Measured end-to-end
rel-err vs the fp64 reference: 1.9e-2 @ N_CORR=2 (gate 2e-2), 1.7e-2 @ 3.

Toolchain constraint baked into the structure: every instruction may carry
at most ONE sync wait (walrus setupSyncWait limit), and Tile's wait elision
is per-proc. Hence (same architecture as the fp32r predecessor):
  - PE "observer" matmuls (tiny, scratch PSUM) absorb each x/weight DMA
    lane wait so real matmuls only wait on the DVE sem (PSUM-bank WAR).
  - Evictions run exclusively on DVE and wait only on PE; out-DMA-slot WAR
    is absorbed by tiny DVE memsets; the bias lane by a tiny DVE copy.
  - x-loads ride the SP HWDGE ring, stores the SWDGE path (own DMASW lane
    per store via one output tensor per super-chunk).
  - A tail chain of 1-dep sync nops covers all procs so the exit drain
    carries at most one wait.
"""
import sys
sys.path.insert(0, "/opt/trn_rl_repo")
import numpy as np
import ml_dtypes

from concourse import bass, mybir, tile
from concourse import bass_utils
from concourse.tile import add_dep_helper

E4M3 = ml_dtypes.float8_e4m3fn

# Problem constants (hardcoded per contract)
B = 8
CI = 256
CO = 256
K = 5
PAD = 2
T = 16384
RANK = 8
ALPHA = 16.0
SCALING = ALPHA / RANK
N_CORES = 8

N_CORR = 2            # correction taps (w-error fix); 2 -> rel_err ~1.9e-2

# Tiling
CHUNK = 1024          # output columns per chunk
NCHUNK = T // CHUNK   # 16
SUB = 512             # matmul free dim (one PSUM bank)
NSUB = CHUNK // SUB   # 2
XCOLS = CHUNK + 2 * PAD  # chunk + halo


def _build_nc(corr_taps, _probe_no_evict=False, _probe_no_store=False):
    f32 = mybir.dt.float32
    f16 = mybir.dt.float16
    f8 = mybir.dt.float8e4
    DR = mybir.MatmulPerfMode.DoubleRow
    n_corr = len(corr_taps)

    nc = bass.Bass(trn_type="TRN2", debug=False)
    # x slots: 0,1 = x_hi(ci blk 0/1); 2,3 = x_lo(ci blk 0/1)
    x = nc.dram_tensor("x", [128, 4, T], f8, kind="ExternalInput").ap()
    wts = nc.dram_tensor("wts", [128, K * 2 * 2 * 128], f8,
                         kind="ExternalInput").ap()
    bias = nc.dram_tensor("bias", [128, 2], f32, kind="ExternalInput").ap()
    zeros = nc.dram_tensor("zeros", [128, 4, PAD], f8, kind="ExternalInput").ap()
    if n_corr:
        wcor = nc.dram_tensor("wcor", [128, n_corr * 2 * 2 * 128], f8,
                              kind="ExternalInput").ap()
    # eight output tensors (one per SWDGE store, fresh DMASW lane each); the
    # last covers only the final 512 columns so the tail transfer is short.
    # Host concatenates along columns.
    Y_COLS = [4 * CHUNK] + [2 * CHUNK] * 5 + [CHUNK + SUB, SUB]
    ys = [nc.dram_tensor(f"y{s}", [128, 2, w], f16, kind="ExternalOutput").ap()
          for s, w in enumerate(Y_COLS)]

    NPB = 6   # psum accumulation banks
    NWARM = 0   # PE warmup matmuls: no-op under TimelineSim's wall-clock
                # p-state model; kept as a knob for real-HW experiments

    with tile.TileContext(nc) as tc:
        with tc.tile_pool(name="wp", bufs=1) as wp, \
             tc.tile_pool(name="pp", bufs=1, space="PSUM") as pp:

            # write-once observer scratch: four columns per observer matmul
            obs_ps = pp.tile([128, 96], f32, name="obs_ps", tag="obs")
            pbufs = [pp.tile([128, SUB], f32, name=f"pt{j}", tag=f"pt{j}")
                     for j in range(NPB)]
            # x is fully resident: one dedicated buffer per chunk, no reuse
            xbufs = [wp.tile([128, 4, XCOLS], f8, name=f"xt{j}", tag=f"xt{j}")
                     for j in range(NCHUNK)]
            # single full-width staging tile: stores slice arbitrary ranges
            ot_all = wp.tile([128, 2, T], f16, name="ot_all")

            if NWARM:
                # PE warmup: junk tile filled by DVE at t0; matmuls on it ramp
                # the PE p-state while the input DMAs stream in.
                junk = wp.tile([128, 2, 256], f8, name="junk")
                wu_ms = nc.vector.memset(junk[:], 0.0)
                for wi in range(NWARM):
                    wm = nc.tensor.matmul(
                        pbufs[0][:, 0:256],
                        junk[:, :, 0:128], junk[:, :, 0:256],
                        start=True, stop=True, perf_mode=DR)
                    if wi == 0:
                        add_dep_helper(wm.ins, wu_ms.ins, sync=True,
                                       reason="warmup")

            wt = wp.tile([128, 2, K, 2, 128], f8, name="wt")
            wview = wts[:].rearrange("p (c k i m) -> p c k i m", c=2, k=K, i=2)
            bs = wp.tile([128, 2], f32, name="bs")

            n_obs = [0]

            def pe_observe(src_ap, dma_inst):
                """Tiny matmul whose only wait is `dma_inst`'s lane.

                Reads only within the region `dma_inst` wrote; writes its own
                never-reused obs_ps columns (no WAW chain)."""
                n = src_ap.shape[-1]
                m = min(4, n)
                oc = 4 * n_obs[0]
                n_obs[0] += 1
                mm = nc.tensor.matmul(obs_ps[0:m, oc:oc + m], src_ap[:, 0:m],
                                      src_ap[:, 0:m], start=True, stop=True)
                add_dep_helper(mm.ins, dma_inst.ins, sync=False,
                               reason="obs-order")
                return mm

            # --- all input DMAs issued upfront (SP HWDGE ring) so stores
            # queue behind them on the shared DMA engines and never delay a
            # load the PE is about to need. Ordered so the first matmul
            # group's deps (co0 weights + chunk-0 first half) land first;
            # later-needed tensors (co1 weights, wcor, bias, chunk-0 second
            # half) follow, each observed just before its first consumer.
            in_dmas = [[] for _ in range(NCHUNK)]
            # cols of chunk 0 needed by its first (ts=0) groups; chosen so
            # both DMA halves have >= 512-byte runs (single-rate DMA)
            HALF0 = SUB + PAD + PAD
            d_w0 = nc.sync.dma_start(out=wt[:, 0], in_=wview[:, 0])
            in_dmas[0].append(nc.sync.dma_start(
                out=xbufs[0][:, :, PAD:HALF0],
                in_=x[:, :, 0:HALF0 - PAD]))
            in_dmas[0].append(nc.sync.dma_start(
                out=xbufs[0][:, :, 0:PAD], in_=zeros[:]))
            d_w1 = nc.sync.dma_start(out=wt[:, 1], in_=wview[:, 1])
            if n_corr:
                wc = wp.tile([128, n_corr, 2, 2, 128], f8, name="wc")
                d_wc = nc.sync.dma_start(
                    out=wc[:],
                    in_=wcor[:].rearrange("p (j c i m) -> p j c i m",
                                          j=n_corr, c=2, i=2))
            d_b = nc.sync.dma_start(out=bs[:], in_=bias[:])
            in_dmas[0].append(nc.sync.dma_start(
                out=xbufs[0][:, :, HALF0:XCOLS],
                in_=x[:, :, HALF0 - PAD:CHUNK + PAD]))
            for c in range(1, NCHUNK):
                lo = c * CHUNK - PAD
                if c == NCHUNK - 1:
                    in_dmas[c].append(nc.sync.dma_start(
                        out=xbufs[c][:, :, 0:CHUNK + PAD],
                        in_=x[:, :, lo:T]))
                    in_dmas[c].append(nc.sync.dma_start(
                        out=xbufs[c][:, :, CHUNK + PAD:XCOLS],
                        in_=zeros[:]))
                else:
                    in_dmas[c].append(nc.sync.dma_start(
                        out=xbufs[c][:], in_=x[:, :, lo:lo + XCOLS]))

            obs_w0 = pe_observe(wt[:, 0, 0, 0], d_w0)
            # deferred observers, emitted just before their first consumer
            pend_w1 = [d_w1]
            pend_wc = [d_wc] if n_corr else []
            # DVE observes the bias lane via a write-once copy
            bscratch = wp.tile([128, 2], f32, name="bscratch")
            obs_b = nc.vector.tensor_copy(bscratch[:], bs[:])

            out_dmas = []     # store DMAs
            last_mm = None
            last_evict = None
            pi = 0            # psum bank rotation

            def emit_store(s):
                # SWDGE store (fresh DMASW lane each): carries only its
                # staging-ready (DVE evict) wait.
                col0 = sum(Y_COLS[:s])
                out_dmas.append(nc.gpsimd.dma_start(
                    out=ys[s][:], in_=ot_all[:, :, col0:col0 + Y_COLS[s]]))

            first_evict = [True]

            def do_group(c, ts, co, seg):
                """One psum accumulation group + eviction. seg: (lo, n) cols
                within the subtile (for the split tail group)."""
                nonlocal last_mm, last_evict, pi
                slo, sn = seg
                xt = xbufs[c]
                pt = pbufs[pi % NPB]
                pi += 1
                mm_specs = (
                    [(wt[:, co, k], 0, k) for k in range(K)] +
                    [(wt[:, co, k], 2, k) for k in range(K)] +
                    [(wc[:, j, co], 0, k) for j, k in enumerate(corr_taps)])
                for n_i, (st, sl, k) in enumerate(mm_specs):
                    if n_i == 10 and pend_wc:
                        observers.append(pe_observe(wc[:, 0, 0, 0],
                                                    pend_wc.pop()))
                    off = ts * SUB + slo + k
                    mm = nc.tensor.matmul(
                        pt[:, 0:sn],
                        st,
                        xt[:, sl:sl + 2, off:off + sn],
                        start=(n_i == 0),
                        stop=(n_i == len(mm_specs) - 1),
                        perf_mode=DR,
                    )
                    while observers:
                        add_dep_helper(mm.ins, observers.pop().ins,
                                       sync=False, reason="order-after-obs")
                    last_mm = mm
                if _probe_no_evict:
                    return
                off = c * CHUNK + ts * SUB + slo
                ev = nc.vector.tensor_scalar(
                    out=ot_all[:, co, off:off + sn],
                    in0=pt[:, 0:sn],
                    scalar1=1.0 / 16.0,
                    scalar2=bs[:, co:co + 1],
                    op0=mybir.AluOpType.mult,
                    op1=mybir.AluOpType.add,
                )
                if first_evict[0]:
                    add_dep_helper(ev.ins, obs_b.ins, sync=False,
                                   reason="order-after-gate")
                    first_evict[0] = False
                last_evict = ev

            observers = []
            for c in range(NCHUNK):
                xt = xbufs[c]

                # PE observes this chunk's x lanes; each observer reads only
                # within its DMA's region. Chunk 0's second-half observer is
                # deferred until its ts=1 groups so ts=0 can start early.
                late_x = []
                for i, d in enumerate(in_dmas[c]):
                    if c == 0:
                        src_ap = [xt[:, 0, PAD:PAD + 4], xt[:, 0, 0:PAD],
                                  xt[:, 0, HALF0:HALF0 + 4]][i]
                    elif c == NCHUNK - 1:
                        src_ap = [xt[:, 0, 0:4],
                                  xt[:, 0, CHUNK + PAD:XCOLS]][i]
                    else:
                        src_ap = xt[:, 0, 0:4]
                    if c == 0 and i == 2:
                        late_x.append((src_ap, d))
                    else:
                        observers.append(pe_observe(src_ap, d))
                if c == 0:
                    observers.append(obs_w0)

                for ts in range(NSUB):
                    if late_x and ts == 1:
                        observers.extend(pe_observe(s, d) for s, d in late_x)
                        late_x = []
                    for co in range(2):
                        if pend_w1 and co == 1:
                            observers.append(pe_observe(wt[:, 1, 0, 0],
                                                        pend_w1.pop()))
                        do_group(c, ts, co, (0, SUB))
                    if c == NCHUNK - 1 and ts == 0 and not _probe_no_evict \
                            and not _probe_no_store:
                        emit_store(6)   # chunk 14 + first half of chunk 15
                if _probe_no_evict or _probe_no_store:
                    continue
                if c in (3, 5, 7, 9, 11, 13):
                    emit_store((3, 5, 7, 9, 11, 13).index(c))

            if not _probe_no_evict and not _probe_no_store:
                emit_store(7)           # final 512 columns

            # Tail flush: cover every proc with 1-dep sync nops so the final
            # drain carries at most one wait.
            tail_deps = [d for ds in in_dmas[-8:] for d in ds] + out_dmas + \
                [last_mm, last_evict]
            for dep in tail_deps:
                if dep is None:
                    continue
                nop = nc.sync.nop()
                add_dep_helper(nop.ins, dep.ins, sync=True, reason="tailflush")

    return nc


def check_waits(nc):
    """Return instructions carrying more than one sync wait (walrus limit)."""
    bad = []
    for f in nc.m.functions:
        for bb in f.blocks:
            for inst in bb.instructions:
                si = inst.sync_info
                nw = len(si.on_wait) if si and si.on_wait else 0
                if nw > 1:
                    bad.append((inst.name, type(inst).__name__, nw,
                                [w.ant_name for w in si.on_wait]))
    return bad


def _q8(a):
    return np.asarray(a, dtype=np.float32).astype(E4M3)


def _pack_weights(conv_w, conv_b, lora_A, lora_B):
    w_eff = (conv_w.astype(np.float64) + SCALING * np.einsum(
        "or,rik->oik", lora_B.astype(np.float64),
        lora_A.astype(np.float64).reshape(RANK, CI, K))).astype(np.float32)
    W16 = _q8(16.0 * w_eff)
    w_lo = w_eff - W16.astype(np.float32) / 16.0
    energies = [(float(np.square(w_lo[:, :, k]).sum()), k) for k in range(K)]
    corr_taps = tuple(sorted(k for _, k in
                             sorted(energies, reverse=True)[:N_CORR]))
    WCOR = _q8(16.0 * w_lo)

    def pack(w8, taps, order):
        a = w8.astype(np.float32).reshape(2, 128, 2, 128, K)  # [c, m, i, p, k]
        a = a[:, :, :, :, list(taps)]                          # [c, m, i, p, j]
        a = a.transpose(order)
        return np.ascontiguousarray(
            a.reshape(128, len(taps) * 2 * 2 * 128)).astype(E4M3)

    # wts[p, c, k, i, m]; wcor[p, j, c, i, m]
    wts = pack(W16, range(K), (3, 0, 4, 2, 1))
    wcor = pack(WCOR, corr_taps, (3, 4, 0, 2, 1)) if N_CORR else None
    bias = np.ascontiguousarray(
        conv_b.astype(np.float32).reshape(2, 128).T)  # [128, 2]
    return wts, wcor, bias, corr_taps


_CACHED = {}


def kernel(x, conv_w, conv_b, lora_A, lora_B, _trace=False):
    x = np.asarray(x, dtype=np.float32)
    wts, wcor, bias, corr_taps = _pack_weights(
        np.asarray(conv_w), np.asarray(conv_b),
        np.asarray(lora_A), np.asarray(lora_B))
    zeros = np.zeros((128, 4, PAD), dtype=E4M3)

    if corr_taps not in _CACHED:
        nc = _build_nc(corr_taps)
        bad = check_waits(nc)
        assert not bad, f"sync-wait violations: {bad[:5]}"
        _CACHED[corr_taps] = nc
    nc = _CACHED[corr_taps]
    # test.py compatibility handle
    kernel.__globals__["_CACHED_NC"] = nc

    x_hi = _q8(x)
    x_lo = _q8(x - x_hi.astype(np.float32))
    # xpack[core][p, s, t]; s = hl*2 + ci_blk
    xp = np.stack([x_hi.reshape(B, 2, 128, T), x_lo.reshape(B, 2, 128, T)],
                  axis=1)                       # [B, hl, blk, p, t]
    xp = np.ascontiguousarray(xp.transpose(0, 3, 1, 2, 4)  # [B, p, hl, blk, t]
                              .reshape(B, 128, 4, T))

    in_maps = []
    for i in range(N_CORES):
        m = {"x": xp[i], "wts": wts, "bias": bias, "zeros": zeros}
        if wcor is not None:
            m["wcor"] = wcor
        in_maps.append(m)
    res = bass_utils.run_bass_kernel_spmd(
        nc, in_maps, core_ids=list(range(N_CORES)), trace=_trace)
    outs = []
    for i in range(N_CORES):
        yc = np.concatenate([np.asarray(res.results[i][f"y{s}"])
                             for s in range(NCHUNK // 2)], axis=2)
        outs.append(yc.transpose(1, 0, 2).reshape(CO, T))
    out = np.stack(outs, axis=0).astype(np.float32)
    if _trace:
        kernel._last_exec_time_ns = res.exec_time_ns
        kernel._last_results = res
    return out


_CACHED_NC = None


if __name__ == "__main__":
    nc = _build_nc((0, 2))
    bad = check_waits(nc)
    print("violations:", bad[:10])
    n_inst = sum(len(bb.instructions) for f in nc.m.functions for bb in f.blocks)
    print("instructions:", n_inst)
    from concourse.timeline_sim import TimelineSim
    dur = TimelineSim(nc, trace=False).simulate()
    print(f"TimelineSim: {dur:.0f} ns")
